# revision 32
# baseline (speedup 1.0000x reference)
"""CosineClassifier Trainium2 kernel (v2, hybrid fp16).

pred[b, c] = (img[b]/||img[b]||) . (concept[b,c]/||concept[b,c]||) / TEMP

Sharding: batch (128) split across 8 cores, 16 samples/core, no comms.

v2 strategy (memory-bound -> halve HBM bytes, then rebalance compute
across DVE/ACT/PE so every engine sits below the new DMA floor):
  - host casts inputs to fp16 (output err ~1e-3 << 2e-2 gate); concept
    HBM traffic per core drops 201MB -> 101MB (floor ~285us @ 358GB/s).
  - classes 0..1023 keep the natural layout [class->part, emb->free]:
    DVE scalar_tensor_tensor dot + ACT Square square-sum per chunk
    (both run at 1 elem/cycle/lane; ~120us each).
  - classes 1024..4095 are host-transposed per sample to [emb, class]
    so the PE can do the work: dot = matmul with a zero-padded
    stationary ([128,32], col b = img slab; zero cols contribute
    exact zeros), square-sum = matmul(ones col 16+b) over elementwise-
    squared slabs (DVE/ACT split).  Out rows 0-15 = dots, 16-31 =
    square-sums, one [32,512] psum bank per 512-class group with a
    single long accumulation group (PE out base partition must be
    32-aligned, hence the zero-padding trick).
  - rsqrt epilogue as exp(-0.5*ln(si*s) + ln(1/TEMP)) on ACT (Ln's
    per-partition scale folds in the |img|^2 factor; Ln/Exp/Square share
    one table set; avoids the slow DVE iterative-divide reciprocal).
    The transposed epilogue runs on [32,512] tiles (all samples at
    once), with one small SBUF->SBUF DMA per group to realign rows
    16-31 onto partitions 0-15.
  - PE instruction stream is software-pipelined by one sample (sample
    b's square-sum matmuls are emitted after sample b+1's dot matmuls)
    so the in-order PE queue never stalls on the elementwise squares.
Measured engine active/core: DMA ~300us, DVE ~231us, ACT ~258us,
PE ~269us -> 337us total (vs 651us baseline).
"""
import sys

for _p in ('/opt/trn_rl_repo',):
    if _p not in sys.path:
        sys.path.insert(0, _p)

import numpy as np

BS, NCLS, D = 128, 4096, 768
NCORES = 8
BPC = BS // NCORES          # samples per core
P = 128
KN = 1024                   # natural-layout classes (per sample)
TN = KN // P                # 8 natural chunks
WT = NCLS - KN              # 3072 transposed classes
GW = 512                    # classes per psum group
NG = WT // GW               # 6 groups
NSL = D // P                # 6 emb slabs
NDBL = NSL // 2             # 3 double-slab DMAs per sample
TEMP = 0.05
T2 = TEMP * TEMP            # Sqrt(T2*q) = TEMP*sqrt(q)

# slab-square engine split: 33 of 48 double-slabs on DVE, rest on ACT
_NDVE = 33
_DVE_SQ = [((i + 1) * _NDVE) // 48 - (i * _NDVE) // 48 == 1 for i in range(48)]
_LN_INV_TEMP = float(np.log(1.0 / TEMP))

_CACHE = {}


def _split_multiwaits(nc, mybir):
    """This toolchain's CoreV3 codegen accepts at most 1 sync-wait per
    instruction (2 for EventSemaphore); Tile sometimes attaches more.
    Move extras onto single-wait NOPs inserted just before, same engine."""
    n = 0
    for f in nc.m.functions:
        for bb in f.blocks:
            il = bb.instructions
            if not any(
                i.sync_info is not None and i.sync_info.on_wait
                and len(i.sync_info.on_wait) > 1 for i in il
            ):
                continue
            out = []
            for inst in il:
                si = inst.sync_info
                cap = 2 if isinstance(inst, mybir.InstEventSemaphore) else 1
                if si is not None and si.on_wait and len(si.on_wait) > cap:
                    waits = list(si.on_wait)
                    for k, w in enumerate(waits[cap:]):
                        out.append(mybir.InstNoOp(
                            name=f"{inst.name}-wsplit{k}",
                            engine=inst.engine,
                            sync_info=mybir.SyncInfo(on_wait=[w], on_update=[]),
                            bass_nofuse=True,
                        ))
                        n += 1
                    si.on_wait = waits[:cap]
                out.append(inst)
            bb.instructions = out
    return n


def _build():
    from concourse import bass, mybir, tile, masks

    f32 = mybir.dt.float32
    f16 = mybir.dt.float16
    Alu = mybir.AluOpType
    Act = mybir.ActivationFunctionType

    nc = bass.Bass("TRN2", target_bir_lowering=False, debug=False, num_devices=1)
    img16 = nc.dram_tensor("img16", [BPC, D], f16, kind="ExternalInput").ap()
    a_nat = nc.dram_tensor(
        "a_nat", [BPC, KN, D], f16, kind="ExternalInput").ap()
    b_tr = nc.dram_tensor(
        "b_tr", [BPC, D, WT], f16, kind="ExternalInput").ap()
    dstat = nc.dram_tensor(
        "dstat", [P, NSL * BPC * 32], f16, kind="ExternalInput").ap()
    qstat = nc.dram_tensor(
        "qstat", [P, BPC * 32], f16, kind="ExternalInput").ap()
    pred = nc.dram_tensor("pred", [BPC, NCLS], f32, kind="ExternalOutput").ap()

    with tile.TileContext(nc) as tc:
        with (
            tc.tile_pool(name="res", bufs=1) as res,
            tc.tile_pool(name="natp", bufs=3) as natp,
            tc.tile_pool(name="imgp", bufs=2) as imgp,
            tc.tile_pool(name="scr", bufs=2) as scr,
            tc.tile_pool(name="dblp", bufs=4) as dblp,
            tc.tile_pool(name="sqp", bufs=6) as sqp,
            tc.tile_pool(name="epi", bufs=2) as epi,
            tc.tile_pool(name="psr", bufs=1,
                         space=bass.MemorySpace.PSUM) as psr,
            tc.tile_pool(name="pst", bufs=2,
                         space=bass.MemorySpace.PSUM) as pst,
        ):
            # ---- persistent tiles ----
            y_nat = res.tile([P, BPC * TN], f32)      # natural dots
            s_nat = res.tile([P, BPC * TN], f32)      # natural |c|^2
            si_nat = res.tile([P, BPC], f32)          # |img|^2 bcast/sample
            imgq = res.tile([32, D], f16)             # img rows twice
            si32 = res.tile([32, 1], f32)             # |img|^2, partition=b
            ds_t = res.tile([P, NSL * BPC * 32], f16)  # dot stationaries
            qs_t = res.tile([P, BPC * 32], f16)       # sq stationaries
            ident = res.tile([P, P], f32)
            masks.make_identity(nc, ident[:])
            lnb = res.tile([P, 1], f32)       # ln(1/TEMP) bias for Exp
            nc.vector.memset(lnb[:], _LN_INV_TEMP)

            nc.sync.dma_start(ds_t[:], dstat)
            nc.sync.dma_start(qs_t[:], qstat)
            nc.sync.dma_start(imgq[0:16, :], img16)
            nc.sync.dma_start(imgq[16:32, :], img16)
            hi_scr = res.tile([32, D], f16)
            nc.scalar.activation(
                hi_scr[:], imgq[:], Act.Square, accum_out=si32[:])

            Tg = [psr.tile([32, GW], f32, name=f"Tg{g}", tag=f"Tg{g}")
                  for g in range(NG)]

            def emit_sq_mms(b, sqs):
                """sq matmuls for sample b (sqs = its 3 sq tiles)."""
                qst = qs_t[:, b * 32:(b + 1) * 32]
                for ds in range(NDBL):
                    for h in range(2):
                        s = ds * 2 + h
                        last = (b == BPC - 1 and s == NSL - 1)
                        for g in range(NG):
                            nc.tensor.matmul(
                                Tg[g][:, :], qst,
                                sqs[ds][:,
                                        h * WT + g * GW:h * WT + (g + 1) * GW],
                                start=False, stop=(last and g == NG - 1))

            dbl_idx = 0
            prev_sqs = None
            for b in range(BPC):
                # ---- DMAs first: natural data, then PE-feeding slabs ----
                imgb = imgp.tile([P, D], f16, tag="imgb")
                nc.sync.dma_start(
                    imgb[:], img16[b:b + 1, :].to_broadcast((P, D)))
                nat = natp.tile([P, TN * D], f16, tag="nat")
                nc.sync.dma_start(
                    nat[:].rearrange("p (t d) -> p t d", t=TN),
                    a_nat[b].rearrange("(t p) d -> p t d", p=P))
                dbls = []
                for ds in range(NDBL):
                    dbl = dblp.tile([P, 2 * WT], f16, tag="dbl")
                    nc.sync.dma_start(
                        dbl[:].rearrange("p (s w) -> p s w", s=2),
                        b_tr[b, ds * 2 * P:(ds + 1) * 2 * P, :]
                        .rearrange("(s p) w -> p s w", p=P))
                    dbls.append(dbl)

                # ---- natural part: classes 0..KN (heads the DVE/ACT
                # queues; slab squares go behind it, they have a full
                # iteration of slack before their matmuls run) ----
                si_scr = scr.tile([P, D], f16, tag="siscr")
                nc.scalar.activation(
                    si_scr[:], imgb[:], Act.Square,
                    accum_out=si_nat[:, b:b + 1])
                for t in range(TN):
                    col = b * TN + t
                    cs = nat[:, t * D:(t + 1) * D]
                    stt_scr = scr.tile([P, D], f16, tag="sttscr")
                    nc.vector.scalar_tensor_tensor(
                        out=stt_scr[:], in0=cs, scalar=1.0, in1=imgb[:],
                        op0=Alu.mult, op1=Alu.mult,
                        accum_out=y_nat[:, col:col + 1])
                    sq_scr = scr.tile([P, D], f16, tag="sqscr")
                    nc.scalar.activation(
                        sq_scr[:], cs, Act.Square,
                        accum_out=s_nat[:, col:col + 1])

                # ---- slab squares on DVE/ACT queues ----
                sqs = []
                for ds in range(NDBL):
                    sq = sqp.tile([P, 2 * WT], f16, tag="sq")
                    if b == BPC - 1:
                        # tail: split across engines, sized by engine speed
                        # (DVE ~0.57 ns/col vs ACT ~0.93), so both finish
                        # together
                        cut = 3840
                        nc.vector.tensor_mul(
                            sq[:, 0:cut], dbls[ds][:, 0:cut],
                            dbls[ds][:, 0:cut])
                        nc.scalar.activation(
                            sq[:, cut:2 * WT], dbls[ds][:, cut:2 * WT],
                            Act.Square)
                    elif _DVE_SQ[dbl_idx]:
                        nc.vector.tensor_mul(sq[:], dbls[ds][:], dbls[ds][:])
                    else:
                        nc.scalar.activation(sq[:], dbls[ds][:], Act.Square)
                    dbl_idx += 1
                    sqs.append(sq)

                # ---- PE: sq matmuls for the PREVIOUS sample first (their
                # inputs are certainly ready -> PE has work while this
                # sample's slabs stream in), then this sample's dots ----
                if prev_sqs is not None:
                    emit_sq_mms(b - 1, prev_sqs)
                prev_sqs = sqs

                for ds in range(NDBL):
                    for h in range(2):
                        s = ds * 2 + h
                        st = ds_t[:, (s * BPC + b) * 32:(s * BPC + b + 1) * 32]
                        for g in range(NG):
                            nc.tensor.matmul(
                                Tg[g][:, :], st,
                                dbls[ds][:,
                                         h * WT + g * GW:h * WT + (g + 1) * GW],
                                start=(b == 0 and s == 0), stop=False)

                # ---- natural epilogue for sample b ----
                yb = y_nat[:, b * TN:(b + 1) * TN]
                sb = s_nat[:, b * TN:(b + 1) * TN]
                # rsqrt(s*si)/TEMP = exp(-0.5*ln(si*s) + ln(1/TEMP)); the
                # si multiply folds into Ln's per-partition scale, and
                # Ln/Exp/Square share one ACT table set (no slow DVE
                # iterative-divide reciprocal, no extra DVE ops).
                rn = epi.tile([P, TN], f32, tag="rn")
                nc.scalar.activation(
                    rn[:], sb, Act.Ln, scale=si_nat[:, b:b + 1])
                nc.scalar.activation(
                    rn[:], rn[:], Act.Exp, bias=lnb[:], scale=-0.5)
                pn = epi.tile([P, TN], f32, tag="pn")
                nc.vector.tensor_mul(pn[:], yb, rn[:])
                pt = pst.tile([TN, P], f32, tag="pt")
                nc.tensor.transpose(pt[:], pn[:], ident[:])
                po = epi.tile([TN, P], f32, tag="po")
                nc.vector.tensor_copy(po[:], pt[:])
                nc.sync.dma_start(
                    pred[b, 0:KN].rearrange("(t p) -> t p", p=P), po[:])

            # ---- last sample's sq matmuls (group-major) interleaved with
            # the per-group transposed epilogue, so epilogue(g) pipelines
            # against group g+1's matmuls ----
            qst_l = qs_t[:, (BPC - 1) * 32:BPC * 32]
            for g in range(NG):
                for ds in range(NDBL):
                    for h in range(2):
                        s = ds * 2 + h
                        nc.tensor.matmul(
                            Tg[g][:, :], qst_l,
                            prev_sqs[ds][:,
                                         h * WT + g * GW:h * WT + (g + 1) * GW],
                            start=False, stop=(s == NSL - 1))
                # rows 0-15 hit Ln(negative) -> NaN; only rows 16-31 are
                # read below, so that's harmless.
                rt = epi.tile([32, GW], f32, tag="rt")
                nc.scalar.activation(
                    rt[:], Tg[g][:, :], Act.Ln, scale=si32[:, 0:1])
                nc.scalar.activation(
                    rt[:], rt[:], Act.Exp, bias=lnb[0:32, :], scale=-0.5)
                rlo = epi.tile([16, GW], f32, tag="rlo")
                nc.sync.dma_start(rlo[:], rt[16:32, :])
                pl = epi.tile([16, GW], f32, tag="pl")
                nc.vector.tensor_mul(pl[:], Tg[g][0:16, :], rlo[:])
                nc.sync.dma_start(
                    pred[:, KN + g * GW:KN + (g + 1) * GW], pl[:])

    _split_multiwaits(nc, mybir)
    return nc


def _get_nc():
    if 'nc' not in _CACHE:
        _CACHE['nc'] = _build()
    return _CACHE['nc']


def kernel(img: np.ndarray, concept: np.ndarray, **run_kwargs) -> np.ndarray:
    from concourse import bass_utils

    img = np.ascontiguousarray(img, dtype=np.float32)
    concept = np.ascontiguousarray(concept, dtype=np.float32)
    assert img.shape == (BS, D) and concept.shape == (BS, NCLS, D)

    img16 = img.astype(np.float16)
    a_nat = np.ascontiguousarray(concept[:, :KN, :].astype(np.float16))
    b_tr = np.ascontiguousarray(
        concept[:, KN:, :].astype(np.float16).transpose(0, 2, 1))

    nc = _get_nc()
    in_maps = []
    for i in range(NCORES):
        sl = slice(i * BPC, (i + 1) * BPC)
        imgc = img16[sl]                               # [16, 768]
        # SBUF image of the zero-padded stationaries, partition-major:
        # dstat[p, (s*BPC+b)*32 + b] = img[b, s*128+p]
        dstat = np.zeros((P, NSL * BPC * 32), np.float16)
        for s in range(NSL):
            for b in range(BPC):
                dstat[:, (s * BPC + b) * 32 + b] = imgc[b, s * P:(s + 1) * P]
        qstat = np.zeros((P, BPC * 32), np.float16)
        for b in range(BPC):
            qstat[:, b * 32 + 16 + b] = 1.0
        in_maps.append({
            "img16": imgc,
            "a_nat": a_nat[sl],
            "b_tr": b_tr[sl],
            "dstat": dstat,
            "qstat": qstat,
        })
    res = bass_utils.run_bass_kernel_spmd(
        nc, in_maps, core_ids=list(range(NCORES)), **run_kwargs)
    out = np.concatenate([r["pred"] for r in res.results], axis=0)
    if run_kwargs:
        _CACHE['last_results'] = res
    return out


# revision 34
# speedup vs baseline: 1.0597x; 1.0597x over previous
"""CosineClassifier Trainium2 kernel (v2, hybrid fp16).

pred[b, c] = (img[b]/||img[b]||) . (concept[b,c]/||concept[b,c]||) / TEMP

Sharding: batch (128) split across 8 cores, 16 samples/core, no comms.

v2 strategy (memory-bound -> halve HBM bytes, then rebalance compute
across DVE/ACT/PE so every engine sits below the new DMA floor):
  - host casts inputs to fp16 (output err ~1e-3 << 2e-2 gate); concept
    HBM traffic per core drops 201MB -> 101MB (floor ~285us @ 358GB/s).
  - classes 0..1023 keep the natural layout [class->part, emb->free]:
    DVE scalar_tensor_tensor dot + ACT Square square-sum per chunk
    (both run at 1 elem/cycle/lane; ~120us each).
  - classes 1024..4095 are host-transposed per sample to [emb, class]
    so the PE can do the work: dot = matmul with a zero-padded
    stationary ([128,32], col b = img slab; zero cols contribute
    exact zeros), square-sum = matmul(ones col 16+b) over elementwise-
    squared slabs (DVE/ACT split).  Out rows 0-15 = dots, 16-31 =
    square-sums, one [32,512] psum bank per 512-class group with a
    single long accumulation group (PE out base partition must be
    32-aligned, hence the zero-padding trick).
  - rsqrt epilogue as exp(-0.5*ln(si*s) + ln(1/TEMP)) on ACT (Ln's
    per-partition scale folds in the |img|^2 factor; Ln/Exp/Square share
    one table set; avoids the slow DVE iterative-divide reciprocal).
    The transposed epilogue runs on [32,512] tiles (all samples at
    once), with one small SBUF->SBUF DMA per group to realign rows
    16-31 onto partitions 0-15.
  - PE instruction stream is software-pipelined by one sample (sample
    b's square-sum matmuls are emitted after sample b+1's dot matmuls)
    so the in-order PE queue never stalls on the elementwise squares.
Measured engine active/core: DMA ~300us, DVE ~231us, ACT ~258us,
PE ~269us -> 337us total (vs 651us baseline).
"""
import sys

for _p in ('/opt/trn_rl_repo',):
    if _p not in sys.path:
        sys.path.insert(0, _p)

import numpy as np

BS, NCLS, D = 128, 4096, 768
NCORES = 8
BPC = BS // NCORES          # samples per core
P = 128
KN = 1024                   # natural-layout classes (per sample)
TN = KN // P                # 8 natural chunks
WT = NCLS - KN              # 3072 transposed classes
GW = 512                    # classes per psum group
NG = WT // GW               # 6 groups
NSL = D // P                # 6 emb slabs
NDBL = NSL // 2             # 3 double-slab DMAs per sample
TEMP = 0.05
T2 = TEMP * TEMP            # Sqrt(T2*q) = TEMP*sqrt(q)

# slab-square engine split: 33 of 48 double-slabs on DVE, rest on ACT
_NDVE = 33
_DVE_SQ = [((i + 1) * _NDVE) // 48 - (i * _NDVE) // 48 == 1 for i in range(48)]
_LN_INV_TEMP = float(np.log(1.0 / TEMP))

_CACHE = {}


def _split_multiwaits(nc, mybir):
    """This toolchain's CoreV3 codegen accepts at most 1 sync-wait per
    instruction (2 for EventSemaphore); Tile sometimes attaches more.
    Move extras onto single-wait NOPs inserted just before, same engine."""
    n = 0
    for f in nc.m.functions:
        for bb in f.blocks:
            il = bb.instructions
            if not any(
                i.sync_info is not None and i.sync_info.on_wait
                and len(i.sync_info.on_wait) > 1 for i in il
            ):
                continue
            out = []
            for inst in il:
                si = inst.sync_info
                cap = 2 if isinstance(inst, mybir.InstEventSemaphore) else 1
                if si is not None and si.on_wait and len(si.on_wait) > cap:
                    waits = list(si.on_wait)
                    for k, w in enumerate(waits[cap:]):
                        out.append(mybir.InstNoOp(
                            name=f"{inst.name}-wsplit{k}",
                            engine=inst.engine,
                            sync_info=mybir.SyncInfo(on_wait=[w], on_update=[]),
                            bass_nofuse=True,
                        ))
                        n += 1
                    si.on_wait = waits[:cap]
                out.append(inst)
            bb.instructions = out
    return n


def _build():
    from concourse import bass, mybir, tile, masks

    f32 = mybir.dt.float32
    f16 = mybir.dt.float16
    Alu = mybir.AluOpType
    Act = mybir.ActivationFunctionType

    nc = bass.Bass("TRN2", target_bir_lowering=False, debug=False, num_devices=1)
    img16 = nc.dram_tensor("img16", [BPC, D], f16, kind="ExternalInput").ap()
    a_nat = nc.dram_tensor(
        "a_nat", [BPC, KN, D], f16, kind="ExternalInput").ap()
    b_tr = nc.dram_tensor(
        "b_tr", [BPC, D, WT], f16, kind="ExternalInput").ap()
    dstat = nc.dram_tensor(
        "dstat", [P, NSL * BPC * 32], f16, kind="ExternalInput").ap()
    qstat = nc.dram_tensor(
        "qstat", [P, BPC * 32], f16, kind="ExternalInput").ap()
    pred = nc.dram_tensor("pred", [BPC, NCLS], f32, kind="ExternalOutput").ap()

    with tile.TileContext(nc) as tc:
        with (
            tc.tile_pool(name="res", bufs=1) as res,
            tc.tile_pool(name="natp", bufs=3) as natp,
            tc.tile_pool(name="imgp", bufs=2) as imgp,
            tc.tile_pool(name="scr", bufs=2) as scr,
            tc.tile_pool(name="dblp", bufs=4) as dblp,
            tc.tile_pool(name="sqp", bufs=6) as sqp,
            tc.tile_pool(name="epi", bufs=2) as epi,
            tc.tile_pool(name="psr", bufs=1,
                         space=bass.MemorySpace.PSUM) as psr,
            tc.tile_pool(name="pst", bufs=2,
                         space=bass.MemorySpace.PSUM) as pst,
        ):
            # ---- persistent tiles ----
            y_nat = res.tile([P, BPC * TN], f32)      # natural dots
            s_nat = res.tile([P, BPC * TN], f32)      # natural |c|^2
            si_nat = res.tile([P, BPC], f32)          # |img|^2 bcast/sample
            imgq = res.tile([32, D], f16)             # img rows twice
            si32 = res.tile([32, 1], f32)             # |img|^2, partition=b
            ds_t = res.tile([P, NSL * BPC * 32], f16)  # dot stationaries
            qs_t = res.tile([P, BPC * 32], f16)       # sq stationaries
            ident = res.tile([P, P], f32)
            masks.make_identity(nc, ident[:])
            lnb = res.tile([P, 1], f32)       # ln(1/TEMP) bias for Exp
            nc.vector.memset(lnb[:], _LN_INV_TEMP)

            hi_scr = res.tile([32, D], f16)

            def setup_dmas():
                # issued between sample 0's natural-data DMAs and its slab
                # DMAs: DVE/ACT start ~5us sooner (imgb+nat land first),
                # while the PE stationaries still arrive before the first
                # dot matmul needs them.
                nc.sync.dma_start(ds_t[:], dstat)
                nc.sync.dma_start(qs_t[:], qstat)
                nc.sync.dma_start(imgq[0:16, :], img16)
                nc.sync.dma_start(imgq[16:32, :], img16)
                nc.scalar.activation(
                    hi_scr[:], imgq[:], Act.Square, accum_out=si32[:])

            Tg = [psr.tile([32, GW], f32, name=f"Tg{g}", tag=f"Tg{g}")
                  for g in range(NG)]

            def emit_sq_mms(b, sqs):
                """sq matmuls for sample b (sqs = its 3 sq tiles)."""
                qst = qs_t[:, b * 32:(b + 1) * 32]
                for ds in range(NDBL):
                    for h in range(2):
                        s = ds * 2 + h
                        last = (b == BPC - 1 and s == NSL - 1)
                        for g in range(NG):
                            nc.tensor.matmul(
                                Tg[g][:, :], qst,
                                sqs[ds][:,
                                        h * WT + g * GW:h * WT + (g + 1) * GW],
                                start=False, stop=(last and g == NG - 1))

            dbl_idx = 0
            prev_sqs = None
            for b in range(BPC):
                # ---- DMAs first: natural data, then PE-feeding slabs ----
                imgb = imgp.tile([P, D], f16, tag="imgb")
                nc.sync.dma_start(
                    imgb[:], img16[b:b + 1, :].to_broadcast((P, D)))
                nat = natp.tile([P, TN * D], f16, tag="nat")
                nc.sync.dma_start(
                    nat[:].rearrange("p (t d) -> p t d", t=TN),
                    a_nat[b].rearrange("(t p) d -> p t d", p=P))
                if b == 0:
                    setup_dmas()
                dbls = []
                for ds in range(NDBL):
                    dbl = dblp.tile([P, 2 * WT], f16, tag="dbl")
                    nc.sync.dma_start(
                        dbl[:].rearrange("p (s w) -> p s w", s=2),
                        b_tr[b, ds * 2 * P:(ds + 1) * 2 * P, :]
                        .rearrange("(s p) w -> p s w", p=P))
                    dbls.append(dbl)

                # ---- natural part: classes 0..KN (heads the DVE/ACT
                # queues; slab squares go behind it, they have a full
                # iteration of slack before their matmuls run) ----
                si_scr = scr.tile([P, D], f16, tag="siscr")
                nc.scalar.activation(
                    si_scr[:], imgb[:], Act.Square,
                    accum_out=si_nat[:, b:b + 1])
                for t in range(TN):
                    col = b * TN + t
                    cs = nat[:, t * D:(t + 1) * D]
                    stt_scr = scr.tile([P, D], f16, tag="sttscr")
                    nc.vector.scalar_tensor_tensor(
                        out=stt_scr[:], in0=cs, scalar=1.0, in1=imgb[:],
                        op0=Alu.mult, op1=Alu.mult,
                        accum_out=y_nat[:, col:col + 1])
                    sq_scr = scr.tile([P, D], f16, tag="sqscr")
                    nc.scalar.activation(
                        sq_scr[:], cs, Act.Square,
                        accum_out=s_nat[:, col:col + 1])

                # ---- slab squares on DVE/ACT queues ----
                sqs = []
                for ds in range(NDBL):
                    sq = sqp.tile([P, 2 * WT], f16, tag="sq")
                    if b == BPC - 1:
                        # tail: split across engines, sized by engine speed
                        # (DVE ~0.57 ns/col vs ACT ~0.93), so both finish
                        # together
                        cut = 3840
                        nc.vector.tensor_mul(
                            sq[:, 0:cut], dbls[ds][:, 0:cut],
                            dbls[ds][:, 0:cut])
                        nc.scalar.activation(
                            sq[:, cut:2 * WT], dbls[ds][:, cut:2 * WT],
                            Act.Square)
                    elif _DVE_SQ[dbl_idx]:
                        nc.vector.tensor_mul(sq[:], dbls[ds][:], dbls[ds][:])
                    else:
                        nc.scalar.activation(sq[:], dbls[ds][:], Act.Square)
                    dbl_idx += 1
                    sqs.append(sq)

                # ---- PE: sq matmuls for the PREVIOUS sample first (their
                # inputs are certainly ready -> PE has work while this
                # sample's slabs stream in), then this sample's dots ----
                if prev_sqs is not None:
                    emit_sq_mms(b - 1, prev_sqs)
                prev_sqs = sqs

                for ds in range(NDBL):
                    for h in range(2):
                        s = ds * 2 + h
                        st = ds_t[:, (s * BPC + b) * 32:(s * BPC + b + 1) * 32]
                        for g in range(NG):
                            nc.tensor.matmul(
                                Tg[g][:, :], st,
                                dbls[ds][:,
                                         h * WT + g * GW:h * WT + (g + 1) * GW],
                                start=(b == 0 and s == 0), stop=False)

                # ---- natural epilogue for sample b ----
                yb = y_nat[:, b * TN:(b + 1) * TN]
                sb = s_nat[:, b * TN:(b + 1) * TN]
                # rsqrt(s*si)/TEMP = exp(-0.5*ln(si*s) + ln(1/TEMP)); the
                # si multiply folds into Ln's per-partition scale, and
                # Ln/Exp/Square share one ACT table set (no slow DVE
                # iterative-divide reciprocal, no extra DVE ops).
                rn = epi.tile([P, TN], f32, tag="rn")
                nc.scalar.activation(
                    rn[:], sb, Act.Ln, scale=si_nat[:, b:b + 1])
                nc.scalar.activation(
                    rn[:], rn[:], Act.Exp, bias=lnb[:], scale=-0.5)
                pn = epi.tile([P, TN], f32, tag="pn")
                nc.vector.tensor_mul(pn[:], yb, rn[:])
                pt = pst.tile([TN, P], f32, tag="pt")
                nc.tensor.transpose(pt[:], pn[:], ident[:])
                po = epi.tile([TN, P], f32, tag="po")
                nc.vector.tensor_copy(po[:], pt[:])
                nc.sync.dma_start(
                    pred[b, 0:KN].rearrange("(t p) -> t p", p=P), po[:])

            # ---- last sample's sq matmuls (group-major) interleaved with
            # the per-group transposed epilogue, so epilogue(g) pipelines
            # against group g+1's matmuls ----
            qst_l = qs_t[:, (BPC - 1) * 32:BPC * 32]
            for g in range(NG):
                for ds in range(NDBL):
                    for h in range(2):
                        s = ds * 2 + h
                        nc.tensor.matmul(
                            Tg[g][:, :], qst_l,
                            prev_sqs[ds][:,
                                         h * WT + g * GW:h * WT + (g + 1) * GW],
                            start=False, stop=(s == NSL - 1))
                # rows 0-15 hit Ln(negative) -> NaN; only rows 16-31 are
                # read below, so that's harmless.
                rt = epi.tile([32, GW], f32, tag="rt")
                nc.scalar.activation(
                    rt[:], Tg[g][:, :], Act.Ln, scale=si32[:, 0:1])
                nc.scalar.activation(
                    rt[:], rt[:], Act.Exp, bias=lnb[0:32, :], scale=-0.5)
                rlo = epi.tile([16, GW], f32, tag="rlo")
                nc.sync.dma_start(rlo[:], rt[16:32, :])
                pl = epi.tile([16, GW], f32, tag="pl")
                nc.vector.tensor_mul(pl[:], Tg[g][0:16, :], rlo[:])
                nc.sync.dma_start(
                    pred[:, KN + g * GW:KN + (g + 1) * GW], pl[:])

    _split_multiwaits(nc, mybir)
    return nc


def _get_nc():
    if 'nc' not in _CACHE:
        _CACHE['nc'] = _build()
    return _CACHE['nc']


def kernel(img: np.ndarray, concept: np.ndarray, **run_kwargs) -> np.ndarray:
    from concourse import bass_utils

    img = np.ascontiguousarray(img, dtype=np.float32)
    concept = np.ascontiguousarray(concept, dtype=np.float32)
    assert img.shape == (BS, D) and concept.shape == (BS, NCLS, D)

    img16 = img.astype(np.float16)
    a_nat = np.ascontiguousarray(concept[:, :KN, :].astype(np.float16))
    b_tr = np.ascontiguousarray(
        concept[:, KN:, :].astype(np.float16).transpose(0, 2, 1))

    nc = _get_nc()
    in_maps = []
    for i in range(NCORES):
        sl = slice(i * BPC, (i + 1) * BPC)
        imgc = img16[sl]                               # [16, 768]
        # SBUF image of the zero-padded stationaries, partition-major:
        # dstat[p, (s*BPC+b)*32 + b] = img[b, s*128+p]
        dstat = np.zeros((P, NSL * BPC * 32), np.float16)
        for s in range(NSL):
            for b in range(BPC):
                dstat[:, (s * BPC + b) * 32 + b] = imgc[b, s * P:(s + 1) * P]
        qstat = np.zeros((P, BPC * 32), np.float16)
        for b in range(BPC):
            qstat[:, b * 32 + 16 + b] = 1.0
        in_maps.append({
            "img16": imgc,
            "a_nat": a_nat[sl],
            "b_tr": b_tr[sl],
            "dstat": dstat,
            "qstat": qstat,
        })
    res = bass_utils.run_bass_kernel_spmd(
        nc, in_maps, core_ids=list(range(NCORES)), **run_kwargs)
    out = np.concatenate([r["pred"] for r in res.results], axis=0)
    if run_kwargs:
        _CACHE['last_results'] = res
    return out
